# revision 3
# baseline (speedup 1.0000x reference)
"""Trainium2 Bass kernel for ContinuousIntegratedKoopmanOperator.

reference: odeint(dz/dt = z @ W) sampled at t = DT*[1..T], y0 = x at t[0].
Closed form (time-invariant linear ODE): out[:, j, :] = x @ expm(DT*j*W).

Strategy:
  host: compute Mj = expm(DT*j*W) for j=0..T-1 in float64; cast x and M
        to fp16 (rel err ~4e-4, tolerance is 2e-2). out[:, 0] = x exactly,
        so the device only computes/writes j=1..63.
  device (8 cores, batch-sharded 1024 rows each):
        out_tile = x @ M_block via single fp16 matmuls (f32 PSUM accum).
        8 batch tiles x 16 j-blocks of 512; PSUM as 2 rotating quad slots
        (4 banks each); drains are 2048-col f32->f16 copies alternating
        Vector/Scalar into triple-buffered fp16 staging; ~1MB half-tile
        DMA outs on the sync HWDGE ring. Loads batched into 5 DMAs.
  sync: raw bass, explicit sems; every wait proves a specific event.
"""
import numpy as np

DT = 0.01
B, D, T = 8192, 128, 64
NCORES = 8
BSH = B // NCORES          # 1024 rows per core
NTILES = BSH // 128        # 8 batch tiles per core
BW = 512                   # j-block width (4 j's of 128)
NBLK = (T * D) // BW       # 16 blocks per tile
QW = 2048                  # drain quad width (4 blocks)
NSTG = 3                   # staging buffers
OW = (T - 1) * D           # 8064 output cols per row (j=1..63)
H0 = 4096 - D              # half-0 width: stg cols [128,4096) -> out [0,3968)
H1 = 4096                  # half-1 width: stg cols [4096,8192) -> out [3968,8064)

_CACHE = {}


def _expm_table(W: np.ndarray) -> np.ndarray:
    """(D, T*D) float64: columns [j*D:(j+1)*D] = expm(DT*j*W)."""
    A = DT * W.astype(np.float64)
    M1 = np.eye(D, dtype=np.float64)
    term = np.eye(D, dtype=np.float64)
    for n in range(1, 24):
        term = term @ A / n
        M1 += term
    Ms = np.empty((T, D, D), dtype=np.float64)
    Ms[0] = np.eye(D)
    for j in range(1, T):
        Ms[j] = Ms[j - 1] @ M1
    return np.ascontiguousarray(Ms.transpose(1, 0, 2).reshape(D, T * D))


def _build_nc():
    import concourse.bass as bass
    import concourse.mybir as mybir

    f16 = mybir.dt.float16

    nc = bass.Bass(trn_type="TRN2")
    xT_d = nc.dram_tensor("xT", (D, NTILES * 128), f16, kind="ExternalInput")
    M_d = nc.dram_tensor("M", (D, NBLK * BW), f16, kind="ExternalInput")
    out_d = nc.dram_tensor("out", (BSH, OW), f16, kind="ExternalOutput")

    xT_s = nc.alloc_sbuf_tensor("xT_s", [D, NTILES * 128], f16)
    M_s = nc.alloc_sbuf_tensor("M_s", [D, NBLK * BW], f16)
    stg = [nc.alloc_sbuf_tensor(f"stg{p}", [128, NBLK * BW], f16) for p in range(NSTG)]
    psum = nc.alloc_psum_tensor("acc", [128, 8 * 512], mybir.dt.float32)

    s_ldx0 = nc.alloc_semaphore("s_ldx0")
    s_ldxr = nc.alloc_semaphore("s_ldxr")
    s_ldm = [nc.alloc_semaphore(f"s_ldm{k}") for k in range(3)]
    s_mm = nc.alloc_semaphore("s_mm")
    s_dv = nc.alloc_semaphore("s_dv")      # Vector drains (quads 0,2)
    s_da = nc.alloc_semaphore("s_da")      # Scalar drains (quads 1,3)
    s_osy = [nc.alloc_semaphore(f"s_osy{p}") for p in range(NSTG)]
    s_boot = nc.alloc_semaphore("s_boot")

    all_sems = [s_ldx0, s_ldxr, *s_ldm, s_mm, s_dv, s_da, *s_osy, s_boot]
    nums = sorted(s.num for s in all_sems)
    assert nums == list(range(nums[0], nums[-1] + 1)), "sems not contiguous"
    sem_range = range(nums[0], nums[-1] + 1)

    nc.gpsimd.dma_reset(sem_range)

    # drain bookkeeping: quad q of tile i -> engine (V for q in {0,2}, S for
    # {1,3}); per-engine count after that quad = 2i + q//2 + 1
    def dr_sem(q):
        return s_dv if q % 2 == 0 else s_da

    def dr_val(i, q):
        return 2 * i + q // 2 + 1

    # number of tiles with index < n mapping to staging p
    def ntile_p(p, n=NTILES):
        return len([i for i in range(n) if i % NSTG == p])

    with nc.Block() as block:
        @block.sync
        def _(sync):
            sync.sem_clear(sem_range)
            sync.nop().then_inc(s_boot, 1)
            # batched loads: tile-0 x + first M blocks first so PE starts ASAP
            sync.dma_start(out=xT_s[:, 0:128], in_=xT_d[:, 0:128]).then_inc(s_ldx0, 16)
            sync.dma_start(out=M_s[:, 0:1024], in_=M_d[:, 0:1024]).then_inc(s_ldm[0], 16)
            sync.dma_start(out=M_s[:, 1024:4096], in_=M_d[:, 1024:4096]).then_inc(s_ldm[1], 16)
            sync.dma_start(out=xT_s[:, 128:], in_=xT_d[:, 128:]).then_inc(s_ldxr, 16)
            sync.dma_start(out=M_s[:, 4096:8192], in_=M_d[:, 4096:8192]).then_inc(s_ldm[2], 16)
            # half-tile outs (~1MB each); j=0 (stg cols 0:128) never written
            for i in range(NTILES):
                p = i % NSTG
                for h in range(2):
                    sync.wait_ge(s_dv, 2 * i + h + 1)
                    sync.wait_ge(s_da, 2 * i + h + 1)
                    if h == 0:
                        sync.dma_start(out=out_d[i * 128:(i + 1) * 128, 0:H0],
                                       in_=stg[p][:, D:D + H0]).then_inc(s_osy[p], 16)
                    else:
                        sync.dma_start(out=out_d[i * 128:(i + 1) * 128, H0:OW],
                                       in_=stg[p][:, 4096:8192]).then_inc(s_osy[p], 16)
            for p in range(NSTG):
                sync.wait_ge(s_osy[p], 32 * ntile_p(p))

        @block.tensor
        def _(tensor):
            tensor.wait_ge(s_boot, 1)
            for i in range(NTILES):
                for b in range(NBLK):
                    q = b // 4                      # quad in tile
                    if i == 0:
                        if b == 0:
                            tensor.wait_ge(s_ldx0, 16)
                            tensor.wait_ge(s_ldm[0], 16)
                        elif b == 2:
                            tensor.wait_ge(s_ldm[1], 16)
                        elif b == 8:
                            tensor.wait_ge(s_ldm[2], 16)
                    if i == 1 and b == 0:
                        tensor.wait_ge(s_ldxr, 16)
                    if b % 4 == 0:
                        Q = 4 * i + q               # global quad
                        if Q >= 2:                  # quad slot reused: drained?
                            i_, q_ = divmod(Q - 2, 4)
                            tensor.wait_ge(dr_sem(q_), dr_val(i_, q_))
                    pb = (q % 2) * 2048 + (b % 4) * 512
                    xt = xT_s[:, i * 128:(i + 1) * 128]
                    mb = M_s[:, b * BW:(b + 1) * BW]
                    tensor.matmul(psum[:, pb:pb + 512], xt, mb,
                                  start=True, stop=True).then_inc(s_mm, 1)

        def drain_stream(eng, parity):
            eng.wait_ge(s_boot, 1)
            for i in range(NTILES):
                p = i % NSTG
                first = True
                for q in range(parity, 4, 2):
                    if first and i >= NSTG:
                        eng.wait_ge(s_osy[p], 32 * ntile_p(p, i - NSTG + 1))
                    first = False
                    eng.wait_ge(s_mm, 16 * i + 4 * (q + 1))  # all 4 blocks of quad
                    pp = (q % 2) * 2048
                    sem = s_dv if parity == 0 else s_da
                    if parity == 0:
                        eng.tensor_copy(out=stg[p][:, q * QW:(q + 1) * QW],
                                        in_=psum[:, pp:pp + QW]).then_inc(sem, 1)
                    else:
                        eng.copy(out=stg[p][:, q * QW:(q + 1) * QW],
                                 in_=psum[:, pp:pp + QW]).then_inc(sem, 1)

        @block.vector
        def _(vector):
            drain_stream(vector, 0)

        @block.scalar
        def _(scalar):
            drain_stream(scalar, 1)

    return nc


def _prep_inputs(x: np.ndarray, Mcat64: np.ndarray):
    """Per-core input maps from the (D, T*D) float64 expm table."""
    Mb = Mcat64.astype(np.float16)
    maps = []
    for c in range(NCORES):
        xc = np.ascontiguousarray(x[c * BSH:(c + 1) * BSH].T.astype(np.float16))
        maps.append({"xT": xc, "M": Mb})
    return maps


def run_on_device(x: np.ndarray, Mcat64: np.ndarray, trace: bool = False):
    from concourse.bass_utils import run_bass_kernel_spmd

    if "nc" not in _CACHE:
        _CACHE["nc"] = _build_nc()
    nc = _CACHE["nc"]

    in_maps = _prep_inputs(x, Mcat64)
    res = run_bass_kernel_spmd(nc, in_maps, core_ids=list(range(NCORES)), trace=trace)
    out = np.empty((B, T, D), dtype=np.float32)
    for c in range(NCORES):
        blk = out[c * BSH:(c + 1) * BSH]
        blk[:, 0, :] = x[c * BSH:(c + 1) * BSH]
        blk[:, 1:, :] = (
            res.results[c]["out"].astype(np.float32).reshape(BSH, T - 1, D))
    return out, res


def kernel(x, W, T):
    x = np.asarray(x, dtype=np.float32)
    W = np.asarray(W, dtype=np.float32)
    assert int(T) == 64 and x.shape == (B, D) and W.shape == (D, D)
    Mcat64 = _expm_table(W)
    out, _ = run_on_device(x, Mcat64, trace=False)
    return out


# revision 7
# speedup vs baseline: 1.2729x; 1.2729x over previous
"""Trainium2 Bass kernel for ContinuousIntegratedKoopmanOperator.

reference: odeint(dz/dt = z @ W) sampled at t = DT*[1..T], y0 = x at t[0].
Closed form (time-invariant linear ODE): out[:, j, :] = x @ expm(DT*j*W).

Strategy:
  host: compute Mj = expm(DT*j*W) for j=0..T-1 in float64; cast x and M
        to fp16 (rel err ~4e-4, tolerance is 2e-2). out[:, 0] = x exactly,
        so the device only computes/writes j=1..63.
  device (8 cores, batch-sharded 1024 rows each):
        out_tile = x @ M_block via single fp16 matmuls (f32 PSUM accum).
        8 batch tiles x 16 j-blocks of 512; PSUM as 2 rotating quad slots
        (4 banks each); drains are 2048-col f32->f16 copies alternating
        Vector/Scalar into triple-buffered fp16 staging; ~1MB half-tile
        DMA outs on the sync HWDGE ring. Loads batched into 5 DMAs.
  sync: raw bass, explicit sems; every wait proves a specific event.
"""
import numpy as np

DT = 0.01
B, D, T = 8192, 128, 64
NCORES = 8
BSH = B // NCORES          # 1024 rows per core
NTILES = BSH // 128        # 8 batch tiles per core
BW = 512                   # j-block width (4 j's of 128)
NBLK = (T * D) // BW       # 16 blocks per tile
NPAIR = 8                  # block-pairs per tile (drain unit = 2 banks)
NSTG = 3                   # staging buffers
OW = (T - 1) * D           # 8064 output cols per row (j=1..63)
H0 = 4096 - D              # half-0 width: stg cols [128,4096) -> out [0,3968)
H1 = 4096                  # half-1 width: stg cols [4096,8192) -> out [3968,8064)

_CACHE = {}


def _expm_table(W: np.ndarray) -> np.ndarray:
    """(D, T*D) float64: columns [j*D:(j+1)*D] = expm(DT*j*W)."""
    A = DT * W.astype(np.float64)
    M1 = np.eye(D, dtype=np.float64)
    term = np.eye(D, dtype=np.float64)
    for n in range(1, 24):
        term = term @ A / n
        M1 += term
    Ms = np.empty((T, D, D), dtype=np.float64)
    Ms[0] = np.eye(D)
    for j in range(1, T):
        Ms[j] = Ms[j - 1] @ M1
    return np.ascontiguousarray(Ms.transpose(1, 0, 2).reshape(D, T * D))


def _build_nc():
    import concourse.bass as bass
    import concourse.mybir as mybir

    f16 = mybir.dt.float16

    nc = bass.Bass(trn_type="TRN2")
    xT_d = nc.dram_tensor("xT", (D, NTILES * 128), f16, kind="ExternalInput")
    M_d = nc.dram_tensor("M", (D, NBLK * BW), f16, kind="ExternalInput")
    out_d = nc.dram_tensor("out", (BSH, OW), f16, kind="ExternalOutput")

    xT_s = nc.alloc_sbuf_tensor("xT_s", [D, NTILES * 128], f16)
    M_s = nc.alloc_sbuf_tensor("M_s", [D, NBLK * BW], f16)
    stg = [nc.alloc_sbuf_tensor(f"stg{p}", [128, NBLK * BW], f16) for p in range(NSTG)]
    psum = nc.alloc_psum_tensor("acc", [128, 8 * 512], mybir.dt.float32)

    s_ldx0 = nc.alloc_semaphore("s_ldx0")
    s_ldxr = nc.alloc_semaphore("s_ldxr")
    s_ldm = [nc.alloc_semaphore(f"s_ldm{k}") for k in range(4)]
    s_mm = nc.alloc_semaphore("s_mm")
    s_dv = nc.alloc_semaphore("s_dv")      # Vector drains (even pairs)
    s_da = nc.alloc_semaphore("s_da")      # Scalar drains (odd pairs)
    s_osy = [nc.alloc_semaphore(f"s_osy{p}") for p in range(NSTG)]
    s_boot = nc.alloc_semaphore("s_boot")

    all_sems = [s_ldx0, s_ldxr, *s_ldm, s_mm, s_dv, s_da, *s_osy, s_boot]
    nums = sorted(s.num for s in all_sems)
    assert nums == list(range(nums[0], nums[-1] + 1)), "sems not contiguous"
    sem_range = range(nums[0], nums[-1] + 1)

    nc.gpsimd.dma_reset(sem_range)

    # drain bookkeeping: pair q of tile i -> engine (V for even q, S odd);
    # per-engine count after that pair = 4i + q//2 + 1
    def dr_sem(q):
        return s_dv if q % 2 == 0 else s_da

    def dr_val(i, q):
        return 4 * i + q // 2 + 1

    # number of tiles with index < n mapping to staging p
    def ntile_p(p, n=NTILES):
        return len([i for i in range(n) if i % NSTG == p])

    with nc.Block() as block:
        @block.sync
        def _(sync):
            sync.sem_clear(sem_range)
            sync.nop().then_inc(s_boot, 1)
            # batched loads: tile-0 x + first M block first so PE starts ASAP,
            # M chunks sized so arrival tracks PE consumption during ramp
            sync.dma_start(out=xT_s[:, 0:128], in_=xT_d[:, 0:128]).then_inc(s_ldx0, 16)
            sync.dma_start(out=M_s[:, 0:512], in_=M_d[:, 0:512]).then_inc(s_ldm[0], 16)
            sync.dma_start(out=M_s[:, 512:2048], in_=M_d[:, 512:2048]).then_inc(s_ldm[1], 16)
            sync.dma_start(out=M_s[:, 2048:5120], in_=M_d[:, 2048:5120]).then_inc(s_ldm[2], 16)
            sync.dma_start(out=xT_s[:, 128:], in_=xT_d[:, 128:]).then_inc(s_ldxr, 16)
            sync.dma_start(out=M_s[:, 5120:8192], in_=M_d[:, 5120:8192]).then_inc(s_ldm[3], 16)
            # half-tile outs (~1MB each); j=0 (stg cols 0:128) never written
            for i in range(NTILES):
                p = i % NSTG
                for h in range(2):
                    sync.wait_ge(s_dv, 4 * i + 2 * (h + 1))
                    sync.wait_ge(s_da, 4 * i + 2 * (h + 1))
                    if h == 0:
                        sync.dma_start(out=out_d[i * 128:(i + 1) * 128, 0:H0],
                                       in_=stg[p][:, D:D + H0]).then_inc(s_osy[p], 16)
                    else:
                        sync.dma_start(out=out_d[i * 128:(i + 1) * 128, H0:OW],
                                       in_=stg[p][:, 4096:8192]).then_inc(s_osy[p], 16)
            for p in range(NSTG):
                sync.wait_ge(s_osy[p], 32 * ntile_p(p))

        @block.tensor
        def _(tensor):
            tensor.wait_ge(s_boot, 1)
            for i in range(NTILES):
                for b in range(NBLK):
                    q = b // 2                      # pair in tile
                    P = i * NPAIR + q               # global pair
                    if i == 0:
                        if b == 0:
                            tensor.wait_ge(s_ldx0, 16)
                            tensor.wait_ge(s_ldm[0], 16)
                        elif b == 1:
                            tensor.wait_ge(s_ldm[1], 16)
                        elif b == 4:
                            tensor.wait_ge(s_ldm[2], 16)
                        elif b == 10:
                            tensor.wait_ge(s_ldm[3], 16)
                    if i == 1 and b == 0:
                        tensor.wait_ge(s_ldxr, 16)
                    if b % 2 == 0 and P >= 4:       # pair slot reused: drained?
                        i_, q_ = divmod(P - 4, NPAIR)
                        tensor.wait_ge(dr_sem(q_), dr_val(i_, q_))
                    pb = (P % 4) * 1024 + (b % 2) * 512
                    xt = xT_s[:, i * 128:(i + 1) * 128]
                    mb = M_s[:, b * BW:(b + 1) * BW]
                    tensor.matmul(psum[:, pb:pb + 512], xt, mb,
                                  start=True, stop=True).then_inc(s_mm, 1)

        def drain_stream(eng, parity):
            eng.wait_ge(s_boot, 1)
            for i in range(NTILES):
                p = i % NSTG
                first = True
                for q in range(parity, NPAIR, 2):
                    P = i * NPAIR + q
                    if first and i >= NSTG:
                        eng.wait_ge(s_osy[p], 32 * ntile_p(p, i - NSTG + 1))
                    first = False
                    eng.wait_ge(s_mm, i * NBLK + 2 * (q + 1))  # both blocks of pair
                    pp = (P % 4) * 1024
                    sem = s_dv if parity == 0 else s_da
                    if parity == 0:
                        eng.tensor_copy(out=stg[p][:, q * 1024:(q + 1) * 1024],
                                        in_=psum[:, pp:pp + 1024]).then_inc(sem, 1)
                    else:
                        eng.copy(out=stg[p][:, q * 1024:(q + 1) * 1024],
                                 in_=psum[:, pp:pp + 1024]).then_inc(sem, 1)

        @block.vector
        def _(vector):
            drain_stream(vector, 0)

        @block.scalar
        def _(scalar):
            drain_stream(scalar, 1)

    return nc


def _prep_inputs(x: np.ndarray, Mcat64: np.ndarray):
    """Per-core input maps from the (D, T*D) float64 expm table."""
    Mb = Mcat64.astype(np.float16)
    maps = []
    for c in range(NCORES):
        xc = np.ascontiguousarray(x[c * BSH:(c + 1) * BSH].T.astype(np.float16))
        maps.append({"xT": xc, "M": Mb})
    return maps


def run_on_device(x: np.ndarray, Mcat64: np.ndarray, trace: bool = False):
    from concourse.bass_utils import run_bass_kernel_spmd

    if "nc" not in _CACHE:
        _CACHE["nc"] = _build_nc()
    nc = _CACHE["nc"]

    in_maps = _prep_inputs(x, Mcat64)
    res = run_bass_kernel_spmd(nc, in_maps, core_ids=list(range(NCORES)), trace=trace)
    out = np.empty((B, T, D), dtype=np.float32)
    for c in range(NCORES):
        blk = out[c * BSH:(c + 1) * BSH]
        blk[:, 0, :] = x[c * BSH:(c + 1) * BSH]
        blk[:, 1:, :] = (
            res.results[c]["out"].astype(np.float32).reshape(BSH, T - 1, D))
    return out, res


def kernel(x, W, T):
    x = np.asarray(x, dtype=np.float32)
    W = np.asarray(W, dtype=np.float32)
    assert int(T) == 64 and x.shape == (B, D) and W.shape == (D, D)
    Mcat64 = _expm_table(W)
    out, _ = run_on_device(x, Mcat64, trace=False)
    return out


# revision 10
# speedup vs baseline: 1.2771x; 1.0034x over previous
"""Trainium2 Bass kernel for ContinuousIntegratedKoopmanOperator.

reference: odeint(dz/dt = z @ W) sampled at t = DT*[1..T], y0 = x at t[0].
Closed form (time-invariant linear ODE): out[:, j, :] = x @ expm(DT*j*W).

Strategy:
  host: compute Mj = expm(DT*j*W) for j=0..T-1 in float64; cast x and M
        to fp16 (rel err ~4e-4, tolerance is 2e-2). out[:, 0] = x exactly,
        so the device only computes/writes j=1..63.
  device (8 cores, batch-sharded 1024 rows each):
        out_tile = x @ M_block via single fp16 matmuls (f32 PSUM accum).
        8 batch tiles x 16 j-blocks of 512; PSUM as 2 rotating quad slots
        (4 banks each); drains are 2048-col f32->f16 copies alternating
        Vector/Scalar into triple-buffered fp16 staging; ~1MB half-tile
        DMA outs on the sync HWDGE ring. Loads batched into 5 DMAs.
  sync: raw bass, explicit sems; every wait proves a specific event.
"""
import numpy as np

DT = 0.01
B, D, T = 8192, 128, 64
NCORES = 8
BSH = B // NCORES          # 1024 rows per core
NTILES = BSH // 128        # 8 batch tiles per core
BW = 512                   # j-block width (4 j's of 128)
NBLK = (T * D) // BW       # 16 blocks per tile
NPAIR = 8                  # block-pairs per tile (drain unit = 2 banks)
NSTG = 3                   # staging buffers
OW = (T - 1) * D           # 8064 output cols per row (j=1..63)
H0 = 4096 - D              # half-0 width: stg cols [128,4096) -> out [0,3968)
H1 = 4096                  # half-1 width: stg cols [4096,8192) -> out [3968,8064)

_CACHE = {}


def _expm_table(W: np.ndarray) -> np.ndarray:
    """(D, T*D) float64: columns [j*D:(j+1)*D] = expm(DT*j*W)."""
    A = DT * W.astype(np.float64)
    M1 = np.eye(D, dtype=np.float64)
    term = np.eye(D, dtype=np.float64)
    for n in range(1, 24):
        term = term @ A / n
        M1 += term
    Ms = np.empty((T, D, D), dtype=np.float64)
    Ms[0] = np.eye(D)
    for j in range(1, T):
        Ms[j] = Ms[j - 1] @ M1
    return np.ascontiguousarray(Ms.transpose(1, 0, 2).reshape(D, T * D))


def _build_nc():
    import concourse.bass as bass
    import concourse.mybir as mybir

    f16 = mybir.dt.float16

    nc = bass.Bass(trn_type="TRN2")
    xT_d = nc.dram_tensor("xT", (D, NTILES * 128), f16, kind="ExternalInput")
    M_d = nc.dram_tensor("M", (D, NBLK * BW), f16, kind="ExternalInput")
    out_d = nc.dram_tensor("out", (BSH, OW), f16, kind="ExternalOutput")

    xT_s = nc.alloc_sbuf_tensor("xT_s", [D, NTILES * 128], f16)
    M_s = nc.alloc_sbuf_tensor("M_s", [D, NBLK * BW], f16)
    stg = [nc.alloc_sbuf_tensor(f"stg{p}", [128, NBLK * BW], f16) for p in range(NSTG)]
    psum = nc.alloc_psum_tensor("acc", [128, 8 * 512], mybir.dt.float32)

    s_ldx0 = nc.alloc_semaphore("s_ldx0")
    s_ldxr = nc.alloc_semaphore("s_ldxr")
    s_ldm = [nc.alloc_semaphore(f"s_ldm{k}") for k in range(4)]
    s_mm = nc.alloc_semaphore("s_mm")
    s_dv = nc.alloc_semaphore("s_dv")      # Vector drains (even pairs)
    s_da = nc.alloc_semaphore("s_da")      # Scalar drains (odd pairs)
    s_osy = [nc.alloc_semaphore(f"s_osy{p}") for p in range(NSTG)]
    s_boot = nc.alloc_semaphore("s_boot")

    all_sems = [s_ldx0, s_ldxr, *s_ldm, s_mm, s_dv, s_da, *s_osy, s_boot]
    nums = sorted(s.num for s in all_sems)
    assert nums == list(range(nums[0], nums[-1] + 1)), "sems not contiguous"
    sem_range = range(nums[0], nums[-1] + 1)

    nc.gpsimd.dma_reset(sem_range)

    # drain bookkeeping: pair q of tile i -> engine (V for even q, S odd);
    # per-engine count after that pair = 4i + q//2 + 1
    def dr_sem(q):
        return s_dv if q % 2 == 0 else s_da

    def dr_val(i, q):
        return 4 * i + q // 2 + 1

    # number of out-DMAs for tiles with index < n mapping to staging p
    # (tile 0 goes out in 5 pieces, later tiles in 2 halves)
    def outs_before(p, n):
        return sum((5 if i == 0 else 2) for i in range(n) if i % NSTG == p)

    with nc.Block() as block:
        @block.sync
        def _(sync):
            sync.sem_clear(sem_range)
            sync.nop().then_inc(s_boot, 1)
            # batched loads: tile-0 x + first M block first so PE starts ASAP,
            # M chunks sized so arrival tracks PE consumption during ramp
            sync.dma_start(out=xT_s[:, 0:128], in_=xT_d[:, 0:128]).then_inc(s_ldx0, 16)
            sync.dma_start(out=M_s[:, 0:512], in_=M_d[:, 0:512]).then_inc(s_ldm[0], 16)
            sync.dma_start(out=M_s[:, 512:2048], in_=M_d[:, 512:2048]).then_inc(s_ldm[1], 16)
            sync.dma_start(out=M_s[:, 2048:5120], in_=M_d[:, 2048:5120]).then_inc(s_ldm[2], 16)
            sync.dma_start(out=xT_s[:, 128:], in_=xT_d[:, 128:]).then_inc(s_ldxr, 16)
            sync.dma_start(out=M_s[:, 5120:8192], in_=M_d[:, 5120:8192]).then_inc(s_ldm[3], 16)
            # outs; j=0 (stg cols 0:128) never written. Tile 0 goes out in
            # per-pair quarters so the write stream starts ASAP; later tiles
            # in ~1MB halves.
            for q in range(4):
                sem = s_dv if q % 2 == 0 else s_da
                sync.wait_ge(sem, q // 2 + 1)
                c0 = max(q * 1024, D)
                sync.dma_start(out=out_d[0:128, c0 - D:(q + 1) * 1024 - D],
                               in_=stg[0][:, c0:(q + 1) * 1024]).then_inc(s_osy[0], 16)
            sync.wait_ge(s_dv, 4)
            sync.wait_ge(s_da, 4)
            sync.dma_start(out=out_d[0:128, H0:OW],
                           in_=stg[0][:, 4096:8192]).then_inc(s_osy[0], 16)
            for i in range(1, NTILES):
                p = i % NSTG
                for h in range(2):
                    sync.wait_ge(s_dv, 4 * i + 2 * (h + 1))
                    sync.wait_ge(s_da, 4 * i + 2 * (h + 1))
                    if h == 0:
                        sync.dma_start(out=out_d[i * 128:(i + 1) * 128, 0:H0],
                                       in_=stg[p][:, D:D + H0]).then_inc(s_osy[p], 16)
                    else:
                        sync.dma_start(out=out_d[i * 128:(i + 1) * 128, H0:OW],
                                       in_=stg[p][:, 4096:8192]).then_inc(s_osy[p], 16)
            for p in range(NSTG):
                sync.wait_ge(s_osy[p], 16 * outs_before(p, NTILES))

        @block.tensor
        def _(tensor):
            tensor.wait_ge(s_boot, 1)
            for i in range(NTILES):
                for b in range(NBLK):
                    q = b // 2                      # pair in tile
                    P = i * NPAIR + q               # global pair
                    if i == 0:
                        if b == 0:
                            tensor.wait_ge(s_ldx0, 16)
                            tensor.wait_ge(s_ldm[0], 16)
                        elif b == 1:
                            tensor.wait_ge(s_ldm[1], 16)
                        elif b == 4:
                            tensor.wait_ge(s_ldm[2], 16)
                        elif b == 10:
                            tensor.wait_ge(s_ldm[3], 16)
                    if i == 1 and b == 0:
                        tensor.wait_ge(s_ldxr, 16)
                    if b % 2 == 0 and P >= 4:       # pair slot reused: drained?
                        i_, q_ = divmod(P - 4, NPAIR)
                        tensor.wait_ge(dr_sem(q_), dr_val(i_, q_))
                    pb = (P % 4) * 1024 + (b % 2) * 512
                    xt = xT_s[:, i * 128:(i + 1) * 128]
                    mb = M_s[:, b * BW:(b + 1) * BW]
                    tensor.matmul(psum[:, pb:pb + 512], xt, mb,
                                  start=True, stop=True).then_inc(s_mm, 1)

        def drain_stream(eng, parity):
            eng.wait_ge(s_boot, 1)
            for i in range(NTILES):
                p = i % NSTG
                first = True
                for q in range(parity, NPAIR, 2):
                    P = i * NPAIR + q
                    if first and i >= NSTG:
                        eng.wait_ge(s_osy[p], 16 * outs_before(p, i - NSTG + 1))
                    first = False
                    eng.wait_ge(s_mm, i * NBLK + 2 * (q + 1))  # both blocks of pair
                    pp = (P % 4) * 1024
                    c0 = D if q == 0 else q * 1024  # j=0 cols never drained
                    sem = s_dv if parity == 0 else s_da
                    if parity == 0:
                        eng.tensor_copy(out=stg[p][:, c0:(q + 1) * 1024],
                                        in_=psum[:, pp + c0 - q * 1024:pp + 1024]).then_inc(sem, 1)
                    else:
                        eng.copy(out=stg[p][:, c0:(q + 1) * 1024],
                                 in_=psum[:, pp + c0 - q * 1024:pp + 1024]).then_inc(sem, 1)

        @block.vector
        def _(vector):
            drain_stream(vector, 0)

        @block.scalar
        def _(scalar):
            drain_stream(scalar, 1)

    return nc


def _prep_inputs(x: np.ndarray, Mcat64: np.ndarray):
    """Per-core input maps from the (D, T*D) float64 expm table."""
    Mb = Mcat64.astype(np.float16)
    maps = []
    for c in range(NCORES):
        xc = np.ascontiguousarray(x[c * BSH:(c + 1) * BSH].T.astype(np.float16))
        maps.append({"xT": xc, "M": Mb})
    return maps


def run_on_device(x: np.ndarray, Mcat64: np.ndarray, trace: bool = False):
    from concourse.bass_utils import run_bass_kernel_spmd

    if "nc" not in _CACHE:
        _CACHE["nc"] = _build_nc()
    nc = _CACHE["nc"]

    in_maps = _prep_inputs(x, Mcat64)
    res = run_bass_kernel_spmd(nc, in_maps, core_ids=list(range(NCORES)), trace=trace)
    out = np.empty((B, T, D), dtype=np.float32)
    for c in range(NCORES):
        blk = out[c * BSH:(c + 1) * BSH]
        blk[:, 0, :] = x[c * BSH:(c + 1) * BSH]
        blk[:, 1:, :] = (
            res.results[c]["out"].astype(np.float32).reshape(BSH, T - 1, D))
    return out, res


def kernel(x, W, T):
    x = np.asarray(x, dtype=np.float32)
    W = np.asarray(W, dtype=np.float32)
    assert int(T) == 64 and x.shape == (B, D) and W.shape == (D, D)
    Mcat64 = _expm_table(W)
    out, _ = run_on_device(x, Mcat64, trace=False)
    return out
